# revision 17
# baseline (speedup 1.0000x reference)
"""Causal squeeze-excite 1d on 8 TRN2 NeuronCores.

Reference computation (per batch b):
    y = causal_ema(x)                      # y[t] = (1-a) y[t-1] + a x[t], y[0] = x[0]
    h = relu(w1 @ y[:, t] + b1)            # (32,)  per time step
    g = sigmoid(w2 @ h + b2)               # (512,) per time step
    out[:, t] = x[:, t] * g
Sharding: data-parallel over batch; core i gets x[2i:2i+2].

Key identities/tricks vs the naive mapping:
  - EMA commutes with the channel projection: w1 @ ema(x) == ema((a*w1) @ x),
    so the DVE scan runs on a 33-row projected sequence, not [512, T].
  - b1/b2 ride a "ones row": w1e column 32 is zero so PSUM row 32 scans
    to 0; relu bias row 32 = 1.0 makes h row 32 a constant 1; w2e row 32
    carries b2, so mm2 emits w2@h + b2 directly and the sigmoid needs no
    bias, letting one ACTIVATE span the 2-bank PSUM tile holding both
    mm2 time-halves (fp32 matmuls must write PSUM partition group 0, so
    batches keep separate 33-row pipelines).
  - The gate multiply runs in-place into the x tile (no separate out
    tile), split DVE / GPSIMD to balance engine load.
  - All loads AND stores issue on the Sync ring (HWDGE); weights/consts
    load via the Scalar ring so the first x load starts at t=0; the
    Scalar engine runs pure compute (relu/sigmoid), keeping the scan
    chain's relu off the store-trigger critical path.  Loads prefetch
    PREF chunks ahead so store triggers never starve the load stream.
"""

import numpy as np
from contextlib import ExitStack

import concourse.bass as bass
import concourse.bacc as bacc
import concourse.tile as tile
import concourse.mybir as mybir
from concourse.bass_utils import run_bass_kernel_spmd

F32 = mybir.dt.float32
F32R = mybir.dt.float32r
BF16 = mybir.dt.bfloat16

N_CORES = 8
B, C, T = 16, 512, 4096
CSQ = 32          # squeeze dim
M1 = CSQ + 1      # mm1 output rows: 32 channels + ones row
P = 128           # SBUF partitions
Tc = 1024         # time chunk per (b, th)
TS = 512          # matmul / scan sub-tile (one PSUM bank)
PREF = 2          # load prefetch distance, in th units
SPL = 3584        # gate-multiply split: DVE [0:SPL), GPSIMD [SPL:4096)
                  # (GPSIMD runs ~4.4ns/col under DVE SBUF-port contention,
                  # so it only gets the final 512 columns off the tail path)


def build_nc(B_loc, cw, C_=C, T_=T):
    d = 1.0 - 1.0 / cw
    NCB = C_ // P      # channel blocks (4)
    # Time chunks: the final 1024 columns run as two 512-col chunks so the
    # last chunk's serial mm1->scan->relu->mm2->sigmoid->gate chain is
    # half as deep (the DMA stream otherwise idles ~6us waiting for it).
    CHUNKS = [(0, 1024), (1024, 1024), (2048, 1024), (3072, 512), (3584, 512)]
    assert sum(c[1] for c in CHUNKS) == T_ and all(
        c % TS == 0 for _, c in CHUNKS)
    NTH = len(CHUNKS)

    nc = bacc.Bacc(trn_type="TRN2")
    x = nc.declare_dram_parameter("x", [B_loc, C_, T_], F32R, isOutput=False)
    w1e = nc.declare_dram_parameter("w1e", [P, NCB * M1], F32R, isOutput=False)
    w2e = nc.declare_dram_parameter("w2e", [M1, C_], F32R, isOutput=False)
    b1e = nc.declare_dram_parameter("b1e", [M1, 1], F32, isOutput=False)
    out = nc.declare_dram_parameter("out", [B_loc, C_, T_], F32, isOutput=True)

    with ExitStack() as ctx:
        tc = ctx.enter_context(tile.TileContext(nc))
        const = ctx.enter_context(tc.tile_pool(name="const", bufs=1))
        # 2*(PREF+1) tiles are live at steady state; +1 slack buffer so a
        # late store completion can't stall the load stream.
        xpool = ctx.enter_context(
            tc.tile_pool(name="xp", bufs=2 * (PREF + 1) + 1))
        gpool = ctx.enter_context(tc.tile_pool(name="gp", bufs=4))
        upool = ctx.enter_context(tc.tile_pool(name="up", bufs=4))
        hpool = ctx.enter_context(tc.tile_pool(name="hp", bufs=4))
        cpool = ctx.enter_context(tc.tile_pool(name="cp", bufs=2))
        php = ctx.enter_context(tc.tile_pool(name="php", bufs=4, space="PSUM"))
        pgp = ctx.enter_context(tc.tile_pool(name="pgp", bufs=2, space="PSUM"))

        # Consts ride the Scalar HWDGE ring so the Sync ring starts on x
        # immediately.
        w1_t = const.tile([P, NCB * M1], F32R, tag="w1e")
        nc.scalar.dma_start(w1_t[:], w1e[:])
        w2_t = const.tile([M1, C_], F32R, tag="w2e")
        nc.scalar.dma_start(w2_t[:], w2e[:])
        b1_t = const.tile([M1, 1], F32, tag="b1e")
        nc.scalar.dma_start(b1_t[:], b1e[:])
        dconst = const.tile([M1, TS], F32, tag="dconst")
        nc.vector.memset(dconst[:], d)

        xv = x.rearrange("b (cb p) t -> b p cb t", p=P)
        ov = out.rearrange("b (cb p) t -> b p cb t", p=P)

        xts = {}

        def emit_loads(ci):
            t0, tcc = CHUNKS[ci]
            tiles = []
            for b in range(B_loc):
                # Full-size tile even for short chunks (cb block stride
                # stays Tc); short chunks just use the first tcc columns
                # of each block.
                xt = xpool.tile([P, NCB * Tc], F32R, tag="x", name=f"x{b}_{ci}")
                xw = xt[:].rearrange("p (cb t) -> p cb t", cb=NCB)
                # 1 MB loads (two channel blocks per DMA): fewer triggers
                # and completion receipts on the Sync ring.
                for cb in range(0, NCB, 2):
                    nc.sync.dma_start(
                        xw[:, cb:cb + 2, 0:tcc],
                        xv[b, :, cb:cb + 2, t0:t0 + tcc])
                tiles.append(xt)
            xts[ci] = tiles

        for ci in range(min(PREF, NTH)):
            emit_loads(ci)

        ph_pre = {}

        def phase1(ci):
            # mm1 for both batches of chunk ci.  Emitted one chunk ahead
            # of the rest of the pipeline so the PE never sits behind a
            # relu-blocked mm2 while independent mm1 work exists.
            _, tcc = CHUNKS[ci]
            nts = tcc // TS
            xws_ = [xt[:].rearrange("p (cb t) -> p cb t", cb=NCB)
                    for xt in xts[ci]]
            phs = [[None] * nts for _ in range(B_loc)]
            for b in range(B_loc):
                for ts in range(nts):
                    ph = php.tile([M1, TS], F32, tag="ph")
                    for cb in range(NCB):
                        nc.tensor.matmul(
                            ph[:],
                            w1_t[:, cb * M1:(cb + 1) * M1],
                            xws_[b][:, cb, ts * TS:(ts + 1) * TS],
                            start=(cb == 0), stop=(cb == NCB - 1))
                    phs[b][ts] = ph
            ph_pre[ci] = phs

        phase1(0)
        carries = [None] * B_loc
        for th in range(NTH):
            if th + PREF < NTH:
                emit_loads(th + PREF)
            if th + 1 < NTH:
                phase1(th + 1)
            t0, tcc = CHUNKS[th]
            nts = tcc // TS
            xb = xts.pop(th)
            xws = [xt[:].rearrange("p (cb t) -> p cb t", cb=NCB) for xt in xb]
            phs = ph_pre.pop(th)
            # Phase 2: scan + relu per batch.
            hts = []
            for b in range(B_loc):
                ut = upool.tile([M1, Tc], F32R, tag="u")
                for ts in range(nts):
                    if th == 0 and ts == 0:
                        # u_0 = cw * p_0 makes y[0] = x[0] exact.
                        init = cpool.tile([M1, 1], F32, tag="c")
                        nc.scalar.mul(init[:], phs[b][ts][:, 0:1], float(cw))
                        init_ap = init[:]
                    else:
                        init_ap = carries[b]
                    nc.vector.tensor_tensor_scan(
                        ut[:, ts * TS:(ts + 1) * TS], dconst[:],
                        phs[b][ts][:], init_ap,
                        mybir.AluOpType.mult, mybir.AluOpType.add)
                    carries[b] = ut[:, (ts + 1) * TS - 1:(ts + 1) * TS]
                # One relu per chunk; bias row 32 = 1.0 plants the ones
                # row that carries b2 through mm2.
                ht = hpool.tile([M1, Tc], F32R, tag="h")
                nc.scalar.activation(
                    ht[:, 0:tcc], ut[:, 0:tcc],
                    mybir.ActivationFunctionType.Relu, bias=b1_t[:])
                hts.append(ht)
            # Phase 3: mm2 + sigmoid per (b, cb); all time sub-tiles of
            # the chunk land in one PSUM tile -> single bias-free sigmoid.
            # G in bf16: halves the gate tile's SBUF footprint; rel err
            # ~2^-9 on a (0,1) gate is well inside the tolerance, and the
            # mixed-dtype gate multiply runs at the same 1x DVE rate.
            gts = [gpool.tile([P, NCB * Tc], BF16, tag="g", name=f"g{b}")
                   for b in range(B_loc)]
            gws = [g[:].rearrange("p (cb t) -> p cb t", cb=NCB) for g in gts]
            for b in range(B_loc):
                for cb in range(NCB):
                    pg = pgp.tile([P, Tc], F32, tag="pg")
                    wsl = w2_t[:, cb * P:(cb + 1) * P]
                    for ts in range(nts):
                        nc.tensor.matmul(
                            pg[:, ts * TS:(ts + 1) * TS], wsl,
                            hts[b][:, ts * TS:(ts + 1) * TS],
                            start=True, stop=True)
                    nc.scalar.activation(
                        gws[b][:, cb, 0:tcc], pg[:, 0:tcc],
                        mybir.ActivationFunctionType.Sigmoid)
            # Phase 4: in-place gate multiply + store.  The multiply is
            # split so the first piece starts after only two sigmoids and
            # GPSIMD (slow under DVE SBUF-port contention) only gets the
            # final tail columns; each store half streams as soon as its
            # half is gated.  Stores stay on the Sync ring with the
            # loads: putting them on the Scalar ring head-of-line blocks
            # relu/sigmoid behind the store trigger's wait (measured
            # +8us).
            for b in range(B_loc):
                if tcc == Tc:
                    # Contiguous column ranges: DVE 7/8, GPSIMD last 1/8.
                    for lo, hi, eng in ((0, 2 * Tc, nc.vector),
                                        (2 * Tc, SPL, nc.vector),
                                        (SPL, NCB * Tc, nc.gpsimd)):
                        eng.tensor_mul(
                            xb[b][:, lo:hi], xb[b][:, lo:hi],
                            gts[b][:, lo:hi])
                else:
                    nc.vector.tensor_mul(
                        xws[b][:, 0:2, 0:tcc], xws[b][:, 0:2, 0:tcc],
                        gws[b][:, 0:2, 0:tcc])
                    nc.vector.tensor_mul(
                        xws[b][:, 2, 0:tcc], xws[b][:, 2, 0:tcc],
                        gws[b][:, 2, 0:tcc])
                    nc.gpsimd.tensor_mul(
                        xws[b][:, 3, 0:tcc], xws[b][:, 3, 0:tcc],
                        gws[b][:, 3, 0:tcc])
                for cb in range(0, NCB, 2):
                    nc.sync.dma_start(
                        ov[b, :, cb:cb + 2, t0:t0 + tcc],
                        xws[b][:, cb:cb + 2, 0:tcc].bitcast(F32))
    nc.compile()
    return nc


def make_in_maps(x, w1, b1, w2, b2, cw, n_cores=N_CORES):
    """Host-side shard + weight prep. Returns per-core input maps."""
    a = 1.0 / cw
    C_ = w2.shape[0]
    NCB = C_ // P
    b_loc = x.shape[0] // n_cores

    w1sT = (np.asarray(w1) * a).T.astype(np.float32)      # [C, CSQ]
    w1e = np.zeros((P, NCB * M1), dtype=np.float32)
    for cb in range(NCB):
        w1e[:, cb * M1:cb * M1 + CSQ] = w1sT[cb * P:(cb + 1) * P, :]
        # column CSQ stays zero -> PSUM ones-row source is 0

    w2e = np.zeros((M1, C_), dtype=np.float32)
    w2e[:CSQ, :] = np.asarray(w2).T
    w2e[CSQ, :] = np.asarray(b2)

    b1e = np.zeros((M1, 1), dtype=np.float32)
    b1e[:CSQ, 0] = np.asarray(b1)
    b1e[CSQ, 0] = 1.0

    return [
        {
            "x": np.ascontiguousarray(x[i * b_loc:(i + 1) * b_loc],
                                      dtype=np.float32),
            "w1e": w1e, "w2e": w2e, "b1e": b1e,
        }
        for i in range(n_cores)
    ]


_NC_CACHE = {}


def kernel(x, w1, b1, w2, b2, context_window):
    cw = int(context_window)
    x = np.asarray(x)
    key = (cw, x.shape)
    if key not in _NC_CACHE:
        _NC_CACHE[key] = build_nc(x.shape[0] // N_CORES, cw)
    nc = _NC_CACHE[key]
    in_maps = make_in_maps(
        np.asarray(x), np.asarray(w1), np.asarray(b1),
        np.asarray(w2), np.asarray(b2), cw)
    res = run_bass_kernel_spmd(nc, in_maps, core_ids=list(range(N_CORES)))
    return np.concatenate([r["out"] for r in res.results], axis=0)
